# revision 2
# baseline (speedup 1.0000x reference)
"""Linear-chain CRF forward loss on 8 Trainium2 NeuronCores.

Math: per (channel, batch) row the reference runs a T=2048-step log-space
scan  alpha_t[j] = logsumexp_i(alpha_{t-1}[i] + trans[i,j]) + em_t[j]  and
returns -(z_sup - z_full).  Rewritten in linear space:

    S_k = (E'^T S_{k-1}) * X'_k        (elementwise in X')

with E' = exp(trans, forbidden->0)/128 in bf16 and X'[k][j,row] =
exp(em - sh)*128 in fp8-e4m3 (channel-0 rows masked by target), where sh is
a per-(t,row) host-side log-growth normalizer added back at the end.  The
128/2^7 scaling centres X' in fp8's normal range and cancels exactly.

Sharding: T is split into 64 chunks (8 per core); products of positive
matrices converge to rank-1 in a few steps, so each chunk reproduces the
true state direction after a short warm-up (W=4).  Per core the 8 chunk
chains run as 2 groups of 4: one [128x128x512] bf16 matmul + one fused
[128,512] DVE multiply per group per round, ping-ponged so PE and DVE
overlap.  Chunk growths are recovered from column-sum probes (ones and
exp(end) stationary) and telescoped on the host in f64.
"""

import math

import numpy as np
import ml_dtypes

import concourse.bacc as bacc
import concourse.mybir as mybir
import concourse.tile as tile
from concourse.bass_utils import run_bass_kernel_spmd

B, T, N = 64, 2048, 128
R = 2 * B
NCORES = 8
G = 2                      # chain groups per core
K = 4                      # chains per group (fused matmul/mult width K*R)
NCHUNK = NCORES * G * K    # 64
W = 4                      # warm-up steps for mid chunks
L = math.ceil((T + (NCHUNK - 1) * W) / NCHUNK)          # 36
KSTAR = (L - W) * (NCHUNK - 2) + L - (T - 1 - L)        # last-chunk probe k
CH = 6                     # X steps per DMA batch tile
FR = K * R                 # fused row width (512)

F32 = mybir.dt.float32
BF16 = mybir.dt.bfloat16
FP8 = mybir.dt.float8e4

NP_BF16 = ml_dtypes.bfloat16
NP_FP8 = ml_dtypes.float8_e4m3

# probe rows in the output tensor: (group, k) -> row
PROBES = [(0, W), (0, L), (1, W), (1, KSTAR), (1, L)]

_COMPILED = {}


def _build_nc():
    if "nc" in _COMPILED:
        return _COMPILED["nc"]

    nc = bacc.Bacc("TRN2", target_bir_lowering=False, debug=False,
                   num_devices=NCORES)

    e_d = nc.dram_tensor("e", [N, N], BF16, kind="ExternalInput").ap()
    oe_d = nc.dram_tensor("oe", [N, 2], BF16, kind="ExternalInput").ap()
    x_ds = [nc.dram_tensor(f"x{g}", [N, L, FR], FP8,
                           kind="ExternalInput").ap() for g in range(G)]
    i_ds = [nc.dram_tensor(f"i{g}", [N, FR], BF16,
                           kind="ExternalInput").ap() for g in range(G)]
    out_d = nc.dram_tensor("probes", [len(PROBES), 2, FR], F32,
                           kind="ExternalOutput").ap()

    nbatch = math.ceil(L / CH)
    probe_rows = {pk: row for row, pk in enumerate(PROBES)}

    with tile.TileContext(nc) as tc:
        with (
            tc.tile_pool(name="consts", bufs=1) as consts,
            tc.tile_pool(name="states", bufs=3) as states,
            tc.tile_pool(name="xtiles", bufs=nbatch) as xtiles,
            tc.tile_pool(name="stage", bufs=len(PROBES)) as stage,
            tc.tile_pool(name="qpsum", bufs=2, space="PSUM") as qpsum,
            tc.tile_pool(name="ppsum", bufs=1, space="PSUM") as ppsum,
        ):
            e_sb = consts.tile([N, N], BF16)
            nc.sync.dma_start(out=e_sb, in_=e_d)
            oe_sb = consts.tile([N, 2], BF16)
            nc.sync.dma_start(out=oe_sb, in_=oe_d)

            S = []
            for g in range(G):
                s0 = states.tile([N, FR], BF16, tag=f"s{g}")
                nc.sync.dma_start(out=s0, in_=i_ds[g])
                S.append(s0)

            # prefetch all X batches (Pool queue: cheap DMA issue)
            xtile = {}
            for kb in range(nbatch):
                for g in range(G):
                    nb = min(CH, L - kb * CH)
                    xt = xtiles.tile([N, nb, FR], FP8, tag=f"x{g}")
                    nc.gpsimd.dma_start(
                        out=xt, in_=x_ds[g][:, kb * CH:kb * CH + nb, :])
                    xtile[(g, kb)] = xt

            def probe(g, s_tile, row):
                p = ppsum.tile([2, FR], F32, tag="p")
                nc.tensor.matmul(p, lhsT=oe_sb, rhs=s_tile,
                                 start=True, stop=True)
                st = stage.tile([2, FR], F32, tag=f"st{row}")
                nc.scalar.copy(out=st, in_=p)
                nc.gpsimd.dma_start(out=out_d[row], in_=st)

            for k in range(1, L + 1):
                for g in range(G):
                    q = qpsum.tile([N, FR], F32, tag=f"q{g}")
                    nc.tensor.matmul(q, lhsT=e_sb, rhs=S[g],
                                     start=True, stop=True)
                    s_new = states.tile([N, FR], BF16, tag=f"s{g}")
                    xk = xtile[(g, (k - 1) // CH)][:, (k - 1) % CH, :]
                    nc.vector.tensor_mul(out=s_new, in0=q, in1=xk)
                    S[g] = s_new
                    if (g, k) in probe_rows:
                        probe(g, s_new, probe_rows[(g, k)])

    nc.compile()
    _COMPILED["nc"] = nc
    return nc


def _host_prep(inputs):
    em = np.asarray(inputs["emissions"], np.float32)
    tgt = np.asarray(inputs["target"])
    trans = np.asarray(inputs["transitions"], np.float32)
    st = np.asarray(inputs["start_transitions"], np.float32)
    en = np.asarray(inputs["end_transitions"], np.float32)
    ft = np.asarray(inputs["forbidden_transitions"]).astype(bool)
    sft = np.asarray(inputs["start_forbidden_transitions"]).astype(bool)
    eft = np.asarray(inputs["end_forbidden_transitions"]).astype(bool)
    mask = np.asarray(inputs["mask"]).astype(bool)
    assert mask.all(), "kernel specialized for all-true mask"

    E = np.where(ft, 0.0, np.exp(trans)).astype(np.float32)
    expst = np.where(sft, 0.0, np.exp(st)).astype(np.float32)
    expen = np.where(eft, 0.0, np.exp(en)).astype(np.float32)

    x1 = np.exp(em.astype(np.float32)).transpose(1, 2, 0)    # [T,N,B]
    x0 = x1 * tgt.astype(np.float32).transpose(1, 2, 0)
    X = np.concatenate([x0, x1], axis=2)                     # [T,N,R]

    Ebar = np.float64(E.astype(np.float64).mean())
    sh = np.log(np.maximum(X.sum(axis=1, dtype=np.float64) * Ebar, 1e-300))
    Xs = (X * (np.exp(-sh)[:, None, :] * 128.0)).astype(np.float32)
    Xq = np.minimum(Xs, np.float32(240.0)).astype(NP_FP8)    # [T,N,R] fp8
    return E, expst, expen, Xq, sh


def kernel(**inputs):
    loss, _ = _run(inputs)
    return loss


def _run(inputs, trace=False, trace_kwargs=None):
    E, expst, expen, Xq, sh = _host_prep(inputs)

    t0s = [(L - W) * j for j in range(NCHUNK - 1)] + [T - 1 - L]

    e_in = np.ascontiguousarray((E * np.float32(1 / 128.0)).astype(NP_BF16))
    oe = np.stack([np.ones(N, np.float32), expen], axis=1)
    oe_in = np.ascontiguousarray(oe.astype(NP_BF16))

    expst_b = expst.astype(NP_BF16).astype(np.float32)

    in_maps = []
    init0 = None
    for core in range(NCORES):
        m = {"e": e_in, "oe": oe_in}
        for g in range(G):
            xg = np.empty((N, L, FR), NP_FP8)
            ig = np.empty((N, FR), NP_BF16)
            for c in range(K):
                j = core * G * K + g * K + c
                t0 = t0s[j]
                sl = slice(c * R, (c + 1) * R)
                xg[:, :, sl] = Xq[t0 + 1:t0 + L + 1].transpose(1, 0, 2)
                if j == 0:
                    i0 = (Xq[0].astype(np.float32)
                          * expst_b[:, None]).astype(NP_BF16)
                    ig[:, sl] = i0
                    init0 = i0.astype(np.float64)
                else:
                    ig[:, sl] = Xq[t0]
            m[f"x{g}"] = np.ascontiguousarray(xg)
            m[f"i{g}"] = np.ascontiguousarray(ig)
        in_maps.append(m)
    cs_init0 = init0.sum(axis=0)                             # [R] f64

    nc = _build_nc()
    kw = {}
    if trace:
        kw["trace"] = True
        if trace_kwargs:
            kw.update(trace_kwargs)
    res = run_bass_kernel_spmd(nc, in_maps, core_ids=list(range(NCORES)), **kw)

    probe_rows = {pk: row for row, pk in enumerate(PROBES)}
    g_log = np.zeros((NCHUNK, R), np.float64)
    for core in range(NCORES):
        outs = res.results[core]["probes"].astype(np.float64)  # [5,2,FR]
        for g in range(G):
            for c in range(K):
                j = core * G * K + g * K + c
                sl = slice(c * R, (c + 1) * R)
                csL = outs[probe_rows[(g, L)], 0, sl]
                if j == 0:
                    g_log[j] = np.log(csL) - np.log(cs_init0)
                elif j == NCHUNK - 1:
                    dot = outs[probe_rows[(g, L)], 1, sl]
                    csK = outs[probe_rows[(g, KSTAR)], 0, sl]
                    g_log[j] = np.log(dot) - np.log(csK)
                else:
                    csW = outs[probe_rows[(g, W)], 0, sl]
                    g_log[j] = np.log(csL) - np.log(csW)

    z = sh.sum(axis=0) + np.log(cs_init0) + g_log.sum(axis=0)
    loss = -(z[:B] - z[B:])
    return loss.astype(np.float32), res
